# revision 63
# baseline (speedup 1.0000x reference)
"""Multi-head attention (BaselineAttention) Bass kernel for 8 trn2 NeuronCores.

Problem: x[4,2048,1024], per-head Wq/Wk/Wv [16,1024,64] (+biases), Wo[1024,1024]+bo.
Sharding: core c -> batch b=c//2, head-group g=c%2 (8 heads each).
Each core computes y_partial[b] = sum_{h in group} softmax(qk^T/8) v @ Wo_rows(h).
Host combines: y[b] = part[2b] + part[2b+1] + bo + bv@Wo  (bv folded out of device).

v2 vs the fp32r baseline:
  - q/k are quantized to fp8e4m3 on the ACT bias-add (quad layout
    [128 = 4 heads x 32 e-low, quad, e-half, S]) so each scores block is ONE
    fp8 DoubleRow matmul (contraction 64 = 32 partitions x 2 free k-tiles,
    0.5 cyc/row) instead of two fp32r matmuls - halves scores tensor time.
    Measured end-to-end rel-err cost of fp8 q/k: ~1.1e-2 (budget 2e-2).
  - exp is split across two engines: ACT computes exact Exp -> bf16, and for
    a subset of tiles DVE computes a bf16 Schraudolph approximation
    (floor(raw*16*log2e + 128*127.043 + .5) as uint16 IS the bf16 bit
    pattern of ~exp(raw/8); the constant-factor bias cancels in softmax,
    the mantissa-linearization wiggle is ~1.7% rms on those tiles only).
  - attn@V and out-proj stay full precision (bf16 att/v, fp32 psum, fp32r
    out-proj): softmax weights in fp8 alone would cost another ~1.1e-2.
  - QKV projections stay fp32r (fp8 x/W costs ~3e-2 - measured, way over).
"""
import numpy as np

B, S, DIM, H, DH = 4, 2048, 1024, 16, 64
NCORES = 8
HPC = H // 2          # heads per core = 8
NPAIR = HPC // 2      # head pairs per core = 4
NQ = HPC // 4         # head quads per core = 2
SCALE = 1.0 / float(np.sqrt(DH))

# exp-engine split: slot index (2*ttg+j) in 0..15 per (pair, s-chunk).
# DVE_SLOTS run a bf16 Schraudolph on DVE straight from PSUM; POOL_SLOTS run
# the same Schraudolph on gpsimd (bit-identical converts) fed by a cheap ACT
# Identity bounce copy (gpsimd has no PSUM port); the rest are exact ACT Exp.
DVE_SLOTS = (1, 5, 8, 11, 13)
POOL_SLOTS = ()   # ACT-bounce latency stalls the chain; keep off
SCH16_A = 16 * 1.44269504   # code = raw*A + C  (raw = q.k, pre-1/8)
SCH16_C = 16249.1387        # calibrated: E[sch/exp]=1 (incl. trunc +.5),
                            # so mixing ACT-exact and DVE-approx tiles in one
                            # softmax row stays unbiased

_CACHE = {}


ATT_FP8_MIXED = False  # fp8-DoubleRow attn@V (with fp8 v + residual
                       # compensation) on ACT-exp tiles; bf16 on DVE tiles


def _build(S=S, DIM=DIM, ncores=NCORES, repeat=1,
           skip_qkv=False, skip_attn=False, skip_oproj=False,
           dve_slots=DVE_SLOTS, pool_slots=POOL_SLOTS, att_fp8=None,
           pipe_depth=2, oproj_interleave=False, extra_slot=-1):
    NT = S // 128
    NSQ = S // 512
    NKT = DIM // 128
    if att_fp8 is None:
        att_fp8 = ATT_FP8_MIXED
    import concourse.bass as bass
    import concourse.mybir as mybir
    import concourse.tile as tile
    from concourse import bacc

    f32 = mybir.dt.float32
    f32r = mybir.dt.float32r
    bf16 = mybir.dt.bfloat16
    f8 = mybir.dt.float8e4
    u16 = mybir.dt.uint16
    AF = mybir.ActivationFunctionType
    Alu = mybir.AluOpType
    DR = mybir.MatmulPerfMode.DoubleRow

    nc = bacc.Bacc("TRN2", target_bir_lowering=False, debug=False,
                   num_devices=ncores)

    x8_d = nc.dram_tensor("x8", [DIM, S], f8, kind="ExternalInput")
    xr_d = nc.dram_tensor("xr", [DIM, S], f8, kind="ExternalInput")
    wq_d = nc.dram_tensor("wq", [DIM, HPC * DH], f8, kind="ExternalInput")
    wqr_d = nc.dram_tensor("wqr", [DIM, HPC * DH], f8, kind="ExternalInput")
    wk_d = nc.dram_tensor("wk", [DIM, HPC * DH], f8, kind="ExternalInput")
    wkr_d = nc.dram_tensor("wkr", [DIM, HPC * DH], f8, kind="ExternalInput")
    wv_d = nc.dram_tensor("wv", [DIM, HPC * DH], f8, kind="ExternalInput")
    wvr_d = nc.dram_tensor("wvr", [DIM, HPC * DH], f8, kind="ExternalInput")
    bq_d = nc.dram_tensor("bq", [128, NPAIR], f32, kind="ExternalInput")
    bk_d = nc.dram_tensor("bk", [128, NPAIR], f32, kind="ExternalInput")
    wo_d = nc.dram_tensor("wo", [HPC * DH, DIM], f32r, kind="ExternalInput")
    y_d = nc.dram_tensor("y", [S, DIM], f32, kind="ExternalOutput")

    with tile.TileContext(nc) as tc:
        with tc.tile_pool(name="persist", bufs=1) as pp:
            # ---- persistent SBUF ----
            qT8 = pp.tile([128, NQ, 2, S], f8)      # [4h*32el, quad, ehalf, s]
            kT8 = pp.tile([128, NQ, 2, S], f8)
            vA = pp.tile([128, HPC, NT, 66], bf16)  # [t%128, h, tt, e|1|pad]
            vA8 = pp.tile([128, HPC, NT, 80], f8,
                          name="vA8") if att_fp8 else None
            vR8 = pp.tile([128, HPC, NT, 80], f8,
                          name="vR8") if att_fp8 else None
            bqs = pp.tile([128, NPAIR], f32)
            bks = pp.tile([128, NPAIR], f32)
            ones_stage = pp.tile([128, HPC * NT], bf16)
            nc.vector.memset(ones_stage, 1.0)
            nc.vector.tensor_copy(
                vA.rearrange("p h t e -> p (h t) e")[:, :, 64:65],
                ones_stage[:, :, None])
            if att_fp8:
                ones8 = pp.tile([128, HPC * NT], f8)
                nc.vector.memset(ones8, 1.0)
                nc.vector.tensor_copy(
                    vA8.rearrange("p h t e -> p (h t) e")[:, :, 64:65],
                    ones8[:, :, None])
                nc.vector.memset(ones8, 0.0)
                nc.vector.tensor_copy(
                    vR8.rearrange("p h t e -> p (h t) e")[:, :, 64:65],
                    ones8[:, :, None])

            for rep in range(repeat):
              # =============== Phase 1: QKV projections (streamed xT) ====
              # x chunks [128,512] stream from DRAM; q/k accumulate in 8
              # parallel 1-bank psums (kt-outer); v is a second stream pass.
              with tc.tile_pool(name=f"qkv{rep}", bufs=1) as qp, \
                   tc.tile_pool(name=f"psA{rep}", bufs=1, space="PSUM") as psA:
                  # q/k projections as compensated fp8 DoubleRow:
                  # x = x8 + xr8, W = W8 + Wr8 (all e4m3); accumulate the
                  # three significant cross terms (x8W8 + x8Wr8 + xr8W8) in
                  # fp32 psum with k=256 per DR step - 25% fewer PE cycles
                  # than fp32r, error ~|xr||Wr| ~ 3e-4 relative (negligible).
                  projs = [] if skip_qkv else [("q", wq_d, wqr_d, qT8, bqs),
                                               ("k", wk_d, wkr_d, kT8, bks)]
                  def _wdma1(key, d_, mt):
                      w = wtiles[(key, mt)]
                      nc.sync.dma_start(
                          out=w,
                          in_=d_.ap().rearrange("(kt p) m -> p kt m", p=128)
                          [:, :, mt * 128:(mt + 1) * 128])

                  wtiles = {}
                  for nm, wd, wrd, dst, bias in projs:
                      for sfx in ("", "r"):
                          for mt in range(NPAIR):
                              wtiles[(nm + sfx, mt)] = qp.tile(
                                  [128, NKT, 128], f8,
                                  name=f"w_{nm}{sfx}{mt}_{rep}",
                                  tag=f"w{nm}{sfx}{mt}", bufs=1)
                  x8_src = x8_d.ap().rearrange("(kt p) s -> p kt s", p=128)
                  xr_src = xr_d.ap().rearrange("(kt p) s -> p kt s", p=128)
                  # per s-chunk: DMA the x8/xr8 chunks once, then a q-pass and
                  # a k-pass; psum tags are double-buffered (4 tags x 2 bufs
                  # = 8 banks) so the ACT drain of chunk N overlaps chunk N+1.
                  for sc in range(NSQ if projs else 0):
                      # cold start: only w(q,0) + the x8 chunk gate the
                      # first matmul; stage everything else behind them
                      if sc == 0 and projs:
                          _wdma1("q", wq_d, 0)
                      x8c = qp.tile([128, NKT, 512], f8, tag="x8c", bufs=2,
                                    name=f"x8c{sc}_{rep}")
                      nc.sync.dma_start(
                          out=x8c, in_=x8_src[:, :, sc * 512:(sc + 1) * 512])
                      if sc == 0 and projs:
                          for mt in range(1, NPAIR):
                              _wdma1("q", wq_d, mt)
                      xrc = qp.tile([128, NKT, 512], f8, tag="xrc", bufs=2,
                                    name=f"xrc{sc}_{rep}")
                      if sc == 0 and projs:
                          for mt in range(NPAIR):
                              _wdma1("qr", wqr_d, mt)
                          nc.sync.dma_start(
                              out=xrc,
                              in_=xr_src[:, :, sc * 512:(sc + 1) * 512])
                          nc.sync.dma_start(out=bqs, in_=bq_d.ap())
                          nc.sync.dma_start(out=bks, in_=bk_d.ap())
                      else:
                          nc.sync.dma_start(
                              out=xrc,
                              in_=xr_src[:, :, sc * 512:(sc + 1) * 512])
                      if sc == 0 and projs:
                          for mt in range(NPAIR):
                              _wdma1("k", wk_d, mt)
                          for mt in range(NPAIR):
                              _wdma1("kr", wkr_d, mt)
                      for nm, wd, wrd, dst, bias in projs:
                          pstiles = {}
                          for mt in range(NPAIR):
                              pstiles[mt] = psA.tile(
                                  [128, 512], f32, name=f"ps{nm}{mt}{sc}_{rep}",
                                  tag=f"ps{mt}", bufs=2)
                          terms = [(nm, x8c), (nm + "r", x8c), (nm, xrc)]
                          for ti, (wkey, xc_) in enumerate(terms):
                              for kp in range(NKT // 2):
                                  for mt in range(NPAIR):
                                      nc.tensor.matmul(
                                          pstiles[mt],
                                          wtiles[(wkey, mt)]
                                          [:, 2 * kp:2 * kp + 2, :],
                                          xc_[:, 2 * kp:2 * kp + 2, :],
                                          start=(ti == 0 and kp == 0),
                                          stop=(ti == 2 and
                                                kp == NKT // 2 - 1),
                                          perf_mode=DR)
                          # bias add + rescale (x was shipped x8, W x64 to
                          # clear e4m3's subnormal range) + fp8 quantize
                          # straight into the quad layout
                          for mt in range(NPAIR):
                              nc.scalar.activation(
                                  dst[:, mt // 2, mt % 2,
                                      sc * 512:(sc + 1) * 512],
                                  pstiles[mt], AF.Identity,
                                  bias=bias[:, mt:mt + 1], scale=1.0 / 512)

                  # ---- v: second streamed pass, same 3-term DR scheme ----
                  wv_sb = qp.tile([128, NKT, HPC * DH], f8, name=f"wvsb_{rep}")
                  nc.sync.dma_start(
                      out=wv_sb,
                      in_=wv_d.ap().rearrange("(kt p) m -> p kt m", p=128))
                  wvr_sb = qp.tile([128, NKT, HPC * DH], f8,
                                   name=f"wvrsb_{rep}")
                  nc.sync.dma_start(
                      out=wvr_sb,
                      in_=wvr_d.ap().rearrange("(kt p) m -> p kt m", p=128))
                  for ttg in ([] if skip_qkv else range(NT // 4)):
                      psv = {}
                      for j4 in range(4):
                          psv[j4] = psA.tile([128, HPC * DH], f32,
                                             name=f"psv{ttg}_{j4}_{rep}",
                                             tag=f"ps{j4}", bufs=2)
                      v8c = qp.tile([128, NKT, 512], f8, tag="x8c", bufs=2,
                                    name=f"v8c{ttg}_{rep}")
                      nc.sync.dma_start(
                          out=v8c, in_=x8_src[:, :, ttg * 512:(ttg + 1) * 512])
                      vrc = qp.tile([128, NKT, 512], f8, tag="xrc", bufs=2,
                                    name=f"vrc{ttg}_{rep}")
                      nc.sync.dma_start(
                          out=vrc, in_=xr_src[:, :, ttg * 512:(ttg + 1) * 512])
                      terms = [(wv_sb, v8c), (wvr_sb, v8c), (wv_sb, vrc)]
                      for ti, (w_, xc_) in enumerate(terms):
                          for kp in range(NKT // 2):
                              for j4 in range(4):
                                  nc.tensor.matmul(
                                      psv[j4],
                                      xc_[:, 2 * kp:2 * kp + 2,
                                          j4 * 128:(j4 + 1) * 128],
                                      w_[:, 2 * kp:2 * kp + 2, :],
                                      start=(ti == 0 and kp == 0),
                                      stop=(ti == 2 and kp == NKT // 2 - 1),
                                      perf_mode=DR)
                      for j4 in range(4):
                          tt = ttg * 4 + j4
                          nc.vector.tensor_scalar(
                              out=vA[:, :, tt, 0:64], in0=psv[j4],
                              scalar1=1.0 / 512, scalar2=None, op0=Alu.mult)
                          if att_fp8:
                              nc.scalar.copy(vA8[:, :, tt, 0:64], psv[j4])
                              nc.vector.tensor_tensor(
                                  out=vR8[:, :, tt, 0:64], in0=psv[j4],
                                  in1=vA8[:, :, tt, 0:64], op=Alu.subtract)

              # ================= Phase 2+3: attention + out-proj =========
              with tc.tile_pool(name=f"att{rep}", bufs=1) as ap_, \
                   tc.tile_pool(name=f"psB{rep}", bufs=1, space="PSUM") as psB:
                  onorm = ap_.tile([128, NPAIR, S], f32r, name=f"onorm_{rep}")
                  wo_sb = ap_.tile([128, NPAIR, DIM], f32r, name=f"wosb_{rep}")
                  nc.sync.dma_start(
                      out=wo_sb,
                      in_=wo_d.ap().rearrange("(p q) m -> q p m", q=128))

                  def _oproj_st(st):
                      # one out-proj s-tile
                      ps = psB.tile([128, DIM], f32, name=f"ps_y{st}_{rep}",
                                    tag="blk", bufs=3)
                      for pp_ in range(NPAIR):
                          lhs = onorm[:, pp_, st * 128:(st + 1) * 128]
                          for nh in range(DIM // 512):
                              nc.tensor.matmul(
                                  ps[:, nh * 512:(nh + 1) * 512],
                                  lhs,
                                  wo_sb[:, pp_, nh * 512:(nh + 1) * 512],
                                  start=(pp_ == 0), stop=(pp_ == NPAIR - 1))
                      ysb = ap_.tile([128, DIM], f32, tag="ysb", bufs=4,
                                     name=f"ysb{st}_{rep}")
                      if st % 2 == 0:
                          nc.vector.tensor_copy(ysb, ps)
                      else:
                          nc.scalar.copy(ysb, ps)
                      nc.sync.dma_start(out=y_d.ap()[st * 128:(st + 1) * 128, :],
                                        in_=ysb)

                  NST_SQ = 512 // 128
                  for sq in ([] if skip_attn else range(NSQ)):
                      for p in range(NPAIR):
                          sqs = slice(sq * 512, (sq + 1) * 512)
                          if sq > 0 and not skip_oproj and oproj_interleave:
                              _oproj_st((sq - 1) * NST_SQ + p)

                          o_ps = [psB.tile([65, 512], f32,
                                           name=f"o{p}_{sq}_{j}_{rep}",
                                           tag=f"o_ps{j}", bufs=1)
                                  for j in range(2)]
                          atts = {}
                          DEPTH = pipe_depth  # attnV trails scores by DEPTH

                          def _emit_scores(ttg):
                              sblk = [psB.tile([128, 2, 512], f32,
                                               name=f"s{p}{sq}{ttg}{j}_{rep}",
                                               tag="blk", bufs=3)
                                      for j in range(2)]
                              dve_j = [
                                  (2 * ttg + j) % 16 in dve_slots
                                  or ((2 * ttg + j) % 16 == extra_slot
                                      and (p * NSQ + sq) % 2 == 0)
                                  for j in range(2)]
                              att = [ap_.tile([128, 2, 512],
                                              bf16 if (dve_j[j] or not att_fp8)
                                              else f8,
                                              name=f"a{p}{sq}{ttg}{j}_{rep}",
                                              tag=f"att{j}{dve_j[j]}", bufs=6)
                                     for j in range(2)]
                              atts[ttg] = (att, dve_j)
                              # scores: one fp8 DoubleRow matmul per t-tile
                              for jj in range(2):
                                  tt = 2 * ttg + jj
                                  for j in range(2):
                                      h = 2 * p + j
                                      bb = 32 * (h % 4)
                                      nc.tensor.matmul(
                                          sblk[j][:, jj, :],
                                          kT8[bb:bb + 32, h // 4, :,
                                              tt * 128:(tt + 1) * 128],
                                          qT8[bb:bb + 32, h // 4, :, sqs],
                                          start=True, stop=True,
                                          perf_mode=DR,
                                          tile_position=(bb, 0))
                              for j in range(2):
                                  slot = (2 * ttg + j) % 16
                                  if dve_j[j]:
                                      nc.vector.tensor_scalar(
                                          out=att[j].bitcast(u16),
                                          in0=sblk[j],
                                          scalar1=SCH16_A, scalar2=SCH16_C,
                                          op0=Alu.mult, op1=Alu.add)
                                  elif slot in pool_slots:
                                      sbf = ap_.tile(
                                          [128, 2, 512], f32, tag=f"sbf{j}",
                                          bufs=2,
                                          name=f"sb{p}{sq}{ttg}{j}_{rep}")
                                      nc.scalar.copy(sbf, sblk[j])
                                      nc.gpsimd.tensor_scalar(
                                          out=att[j].bitcast(u16),
                                          in0=sbf,
                                          scalar1=SCH16_A, scalar2=SCH16_C,
                                          op0=Alu.mult, op1=Alu.add)
                                  else:
                                      nc.scalar.activation(
                                          att[j], sblk[j], AF.Exp, scale=SCALE)

                          def _emit_attnv(ttg):
                              att, dve_j = atts.pop(ttg)
                              for j in range(2):
                                  h = 2 * p + j
                                  if att_fp8 and not dve_j[j]:
                                      nc.tensor.matmul(
                                          o_ps[j],
                                          vA8[:, h, 2 * ttg:2 * ttg + 2, 0:65],
                                          att[j],
                                          start=(ttg == 0), stop=False,
                                          perf_mode=DR)
                                      nc.tensor.matmul(
                                          o_ps[j],
                                          vR8[:, h, 2 * ttg:2 * ttg + 2, 0:65],
                                          att[j],
                                          start=False,
                                          stop=(ttg == NT // 2 - 1),
                                          perf_mode=DR)
                                  else:
                                      for jj in range(2):
                                          tt = 2 * ttg + jj
                                          nc.tensor.matmul(
                                              o_ps[j],
                                              vA[:, h, tt, 0:65],
                                              att[j][:, jj, :],
                                              start=(ttg == 0 and jj == 0),
                                              stop=(ttg == NT // 2 - 1
                                                    and jj == 1))

                          for ttg in range(NT // 2 + DEPTH):
                              if ttg < NT // 2:
                                  _emit_scores(ttg)
                              if ttg >= DEPTH:
                                  _emit_attnv(ttg - DEPTH)
                          # ---- normalize ----
                          # One quick DVE copy frees the o_ps accumulator
                          # bank for the next (p,sq); the rest of the
                          # normalization runs off SBUF (mults on gpsimd,
                          # which has no PSUM port but plenty of idle time).
                          for j in range(2):
                              ocp = ap_.tile([65, 512], f32, tag=f"ocp{j}",
                                             bufs=3, name=f"oc{p}{sq}{j}_{rep}")
                              nc.vector.tensor_copy(ocp, o_ps[j])
                              rinv = ap_.tile([1, 512], f32, tag=f"ri{j}",
                                              bufs=2, name=f"ri{p}{sq}{j}_{rep}")
                              nc.vector.reciprocal(rinv, ocp[64:65, :])
                              rbc = ap_.tile([64, 512], f32, tag=f"rb{j}",
                                             bufs=2, name=f"rb{p}{sq}{j}_{rep}")
                              nc.gpsimd.partition_broadcast(rbc, rinv)
                              nc.gpsimd.tensor_tensor(
                                  out=onorm[64 * j:64 * j + 64, p, sqs],
                                  in0=ocp[0:64, :], in1=rbc,
                                  op=Alu.mult)

                  # ---- out-projection (tiles not already interleaved) --
                  if not skip_oproj:
                      st0 = (NSQ - 1) * NST_SQ if (
                          oproj_interleave and not skip_attn) else 0
                      for st in range(st0, NT):
                          _oproj_st(st)
    nc.compile()
    return nc


def _get_nc():
    if "nc" not in _CACHE:
        _CACHE["nc"] = _build()
    return _CACHE["nc"]


def _wqk_perm(W):
    """[8, DIM, 64] head-stacked -> [DIM, 512] quad-permuted layout.

    Column mt*128 + hq*32 + el  <-  head 4*(mt//2)+hq, e = 32*(mt%2)+el,
    so projection psum m-tiles land directly in the DoubleRow quad layout.
    """
    a = W.reshape(NQ, 4, DIM, 2, 32)          # [q2, hq, d, eh, el]
    a = a.transpose(2, 0, 3, 1, 4)            # [d, q2, eh, hq, el]
    return np.ascontiguousarray(a.reshape(DIM, HPC * DH))


def _bqk_perm(b):
    """[8, 64] head-stacked biases -> [128, 4] per-m-tile columns."""
    a = b.reshape(NQ, 4, 2, 32).transpose(0, 2, 1, 3)   # [q2, eh, hq, el]
    return np.ascontiguousarray(a.reshape(NPAIR, 128).T)


def _split8(a, scale):
    """fp8 main + fp8 residual of a float32 array, pre-scaled by a power of
    two so both land in e4m3's normal range (undone exactly on device)."""
    import ml_dtypes
    f8 = ml_dtypes.float8_e4m3
    a = np.ascontiguousarray(a) * np.float32(scale)
    a8 = a.astype(f8)
    ar = (a - a8.astype(np.float32)).astype(f8)
    return a8, ar


def make_in_maps(x, Wq, Wk, Wv, bq, bk, bv, Wo, bo):
    in_maps = []
    for c in range(NCORES):
        b, g = c // 2, c % 2
        hs = slice(g * HPC, (g + 1) * HPC)
        x8, xr = _split8(x[b].T, 8)
        wq8, wqr = _split8(_wqk_perm(Wq[hs]), 64)
        wk8, wkr = _split8(_wqk_perm(Wk[hs]), 64)
        wv8, wvr = _split8(
            Wv[hs].transpose(1, 0, 2).reshape(DIM, HPC * DH), 64)
        in_maps.append({
            "x8": x8, "xr": xr,
            "wq": wq8, "wqr": wqr,
            "wk": wk8, "wkr": wkr,
            "wv": wv8, "wvr": wvr,
            "bq": _bqk_perm(bq[hs]),
            "bk": _bqk_perm(bk[hs]),
            "wo": np.ascontiguousarray(Wo[g * 512:(g + 1) * 512, :]),
        })
    return in_maps


def combine(results, bv, Wo, bo):
    const = bv.reshape(DIM) @ Wo + bo          # [DIM]
    y = np.empty((B, S, DIM), dtype=np.float32)
    for b in range(B):
        y[b] = results[2 * b]["y"] + results[2 * b + 1]["y"] + const
    return y


def kernel(x, Wq, Wk, Wv, bq, bk, bv, Wo, bo):
    import time
    from concourse.bass_utils import run_bass_kernel_spmd
    x, Wq, Wk, Wv, bq, bk, bv, Wo, bo = [
        np.asarray(a, dtype=np.float32)
        for a in (x, Wq, Wk, Wv, bq, bk, bv, Wo, bo)]
    nc = _get_nc()
    in_maps = make_in_maps(x, Wq, Wk, Wv, bq, bk, bv, Wo, bo)
    last = None
    for attempt in range(3):
        try:
            res = run_bass_kernel_spmd(nc, in_maps,
                                       core_ids=list(range(NCORES)))
            return combine(res.results, bv, Wo, bo)
        except Exception as e:  # transient NRT_EXEC_UNIT_UNRECOVERABLE wedges
            last = e
            time.sleep(75)
    raise last
